# revision 17
# baseline (speedup 1.0000x reference)
"""BSMatchStar Trainium2 kernel (v3, software-pipelined).

out = (a | (((a&b) +_brev b) ^ b))  -- bitstream MatchStar via a 2^29-bit
big-integer addition over per-byte bit-reversed operands.

Layout: per core, the 8 MiB byte-slice is uint32 words [128, 16384];
each partition owns a contiguous 16384-word chunk of the stream. Tiles
slice the free dim [128, F]. The limb-carry scan chains across tiles
via a [128,1] running state (seed 2.0 marks the all-propagate prefix),
so there is no cross-partition stitch on device; row-boundary carries
are patched on the host by walking the all-propagate prefix of each row
(expected ~1 byte per row).

Software pipeline: phase A(t) = load, a&b, brev x2, s = tr+br (gpsimd);
phase B(t) = flags, scan, carry apply (gpsimd), output brev, store.
Emitted as A(0) A(1) B(0) A(2) B(1) ... so the two mandatory gpsimd
adds always overlap DVE work from the neighboring phase.

Engines: DVE does all bitwise/shift work (gpsimd supports only int add
for uint32; CCE/ACT can't do bitwise). ACT computes the scan flags
p8 = relu(1-2*~s) and g01 = sign(msb-bit) off the DVE's back.
"""
import sys
sys.path.insert(0, "/opt/trn_rl_repo")

import numpy as np

N_BYTES = 67_108_864
N_CORES = 8
P = 128
F = 2048
WORDS_PER_CORE = N_BYTES // 4 // N_CORES   # 2,097,152
ROW_WORDS = WORDS_PER_CORE // P            # 16,384
T = ROW_WORDS // F                         # 8
ROW_BYTES = ROW_WORDS * 4                  # 65,536

_BREV = np.array([int(f"{i:08b}"[::-1], 2) for i in range(256)], dtype=np.uint8)

_cache = {}

ML = [0xF0F0F0F0, 0xCCCCCCCC, 0xAAAAAAAA]
MR = [0x0F0F0F0F, 0x33333333, 0x55555555]
SH = [4, 2, 1]


def _build(n_tiles, f):
    import concourse.bacc as bacc
    import concourse.tile as tile
    import concourse.mybir as mybir
    import contextlib

    AOT = mybir.AluOpType
    AFT = mybir.ActivationFunctionType
    dt = mybir.dt

    nc = bacc.Bacc("TRN2", target_bir_lowering=False, debug=False)

    nf = n_tiles * f
    d_a = nc.dram_tensor("a", [P, nf], dt.uint32, kind="ExternalInput")
    d_b = nc.dram_tensor("b", [P, nf], dt.uint32, kind="ExternalInput")
    d_o = nc.dram_tensor("o", [P, nf], dt.uint32, kind="ExternalOutput")
    d_rs = nc.dram_tensor("rowst", [P, 1], dt.int8, kind="ExternalOutput")

    with tile.TileContext(nc) as tc, contextlib.ExitStack() as ctx:
        pool = ctx.enter_context(tc.tile_pool(name="sb", bufs=1))
        iop = ctx.enter_context(tc.tile_pool(name="io", bufs=2))
        smp = ctx.enter_context(tc.tile_pool(name="sm", bufs=2))

        state0 = smp.tile([P, 1], dt.float32, tag="state")
        nc.vector.memset(state0[:], 0.0)
        allprop0 = smp.tile([P, 1], dt.float32, tag="allprop")
        nc.vector.memset(allprop0[:], 1.0)

        def stt(out, in0, imm, in1, op0, op1):
            nc.vector.add_instruction(
                mybir.InstTensorScalarPtr(
                    name=nc.get_next_instruction_name(),
                    is_scalar_tensor_tensor=True,
                    op0=op0, op1=op1,
                    ins=[nc.vector.lower_ap(in0),
                         mybir.ImmediateValue(dtype=dt.uint32, value=imm),
                         nc.vector.lower_ap(in1)],
                    outs=[nc.vector.lower_ap(out)],
                )
            )

        def brev(dst_tag, src, ts, dst_bufs=1, comb=None):
            x = src
            for li in range(3):
                A = pool.tile([P, f], dt.uint32, tag=f"{ts}A", name=f"{ts}A")
                B = pool.tile([P, f], dt.uint32, tag=f"{ts}B", name=f"{ts}B")
                nc.vector.tensor_scalar(A[:], x[:], SH[li], ML[li],
                                        AOT.logical_shift_left,
                                        AOT.bitwise_and)
                nc.vector.tensor_scalar(B[:], x[:], SH[li], MR[li],
                                        AOT.logical_shift_right,
                                        AOT.bitwise_and)
                y = pool.tile([P, f], dt.uint32,
                              tag=(dst_tag if li == 2 else f"{ts}Y"),
                              name=f"{ts}y{li}",
                              bufs=(dst_bufs if li == 2 else None))
                eng = comb[li] if comb else nc.vector
                op = AOT.add if eng is nc.gpsimd else AOT.bitwise_or
                eng.tensor_tensor(y[:], A[:], B[:], op)
                x = y
            return x

        ctxs = {}

        def phaseA(t):
            sl = slice(t * f, (t + 1) * f)
            a_t = iop.tile([P, f], dt.uint32, tag="a_t")
            b_t = iop.tile([P, f], dt.uint32, tag="b_t")
            nc.sync.dma_start(a_t[:], d_a[:, sl])
            nc.sync.dma_start(b_t[:], d_b[:, sl])

            t0 = pool.tile([P, f], dt.uint32, tag="vY", name="t0")
            nc.vector.tensor_tensor(t0[:], a_t[:], b_t[:], AOT.bitwise_and)

            tr = brev("tr", t0, "v", dst_bufs=2,
                      comb=[nc.gpsimd, nc.vector, nc.vector])
            br = brev("br", b_t, "u", dst_bufs=2)

            s = pool.tile([P, f], dt.uint32, tag="s", bufs=2)
            nc.gpsimd.tensor_tensor(s[:], tr[:], br[:], AOT.add)
            ctxs[t] = (a_t, tr, br, s)

        def phaseB(t, state, allprop):
            sl = slice(t * f, (t + 1) * f)
            a_t, tr, br, s = ctxs.pop(t)

            nots = pool.tile([P, f], dt.uint32, tag="nots")
            nc.vector.tensor_scalar(nots[:], s[:], 0xFFFFFFFF, None,
                                    AOT.bitwise_xor)
            # p8 = (nots == 0) via relu(1 - 2*nots) on the Scalar engine;
            # accum_out = row sum of p8 (for the all-propagate row flag)
            p8 = pool.tile([P, f], dt.int8, tag="p8")
            psum = smp.tile([P, 1], dt.float32, tag="psum")
            nc.scalar.activation(p8[:], nots[:], AFT.Relu,
                                 bias=1.0, scale=-2.0, accum_out=psum[:])
            # gmsb = msb & (tr | (br & ~s)), fused into two stt ops
            n1m = pool.tile([P, f], dt.uint32, tag="n1")
            stt(n1m[:], nots[:], 0x80000000, br[:],
                AOT.bitwise_and, AOT.bitwise_and)
            gmsb = pool.tile([P, f], dt.uint32, tag="g2")
            stt(gmsb[:], tr[:], 0x80000000, n1m[:],
                AOT.bitwise_and, AOT.bitwise_or)
            # g01 = sign(gmsb) in {0,1} on the Scalar engine
            g01 = pool.tile([P, f], dt.int8, tag="g01")
            nc.scalar.activation(g01[:], gmsb[:], AFT.Sign)

            # inclusive scan (uint32 out) into cols 1..f; col 0 = incoming
            # state, so lcb[:, 0:f] is the exclusive carry-in per limb,
            # directly usable as the +carry operand
            lcb = pool.tile([P, f + 1], dt.uint32, tag="lcb")
            nc.vector.tensor_copy(lcb[:, 0:1], state[:])
            nc.vector.tensor_tensor_scan(lcb[:, 1:f + 1], p8[:], g01[:],
                                         state[:], AOT.mult, AOT.max)
            nstate = smp.tile([P, 1], dt.float32, tag="state", name="nstate")
            nc.vector.tensor_copy(nstate[:], lcb[:, f:f + 1])

            # all-propagate row tracking: allprop *= (sum(p8) == f)
            rprop = smp.tile([P, 1], dt.float32, tag="rprop")
            nc.vector.tensor_scalar(rprop[:], psum[:], float(f), None,
                                    AOT.is_equal)
            nallprop = smp.tile([P, 1], dt.float32, tag="allprop",
                                name="nallprop")
            nc.vector.tensor_tensor(nallprop[:], allprop[:], rprop[:],
                                    AOT.mult)

            sp = pool.tile([P, f], dt.uint32, tag="nots", name="sp")
            nc.gpsimd.tensor_tensor(sp[:], s[:], lcb[:, 0:f], AOT.add)
            w = pool.tile([P, f], dt.uint32, tag="n1", name="w")
            nc.vector.tensor_tensor(w[:], sp[:], br[:], AOT.bitwise_xor)

            wb = brev("wY", w, "w")
            o_t = iop.tile([P, f], dt.uint32, tag="o_t", bufs=2)
            nc.vector.tensor_tensor(o_t[:], wb[:], a_t[:], AOT.bitwise_or)
            nc.sync.dma_start(d_o[:, sl], o_t[:])
            return nstate, nallprop

        state, allprop = state0, allprop0
        phaseA(0)
        for t in range(1, n_tiles):
            phaseA(t)
            state, allprop = phaseB(t - 1, state, allprop)
        state, allprop = phaseB(n_tiles - 1, state, allprop)

        # rowstate = carry-out {0,1} + 2*allprop (mutually exclusive)
        rsf = smp.tile([P, 1], dt.float32, tag="rsf")
        nc.vector.tensor_scalar(rsf[:], allprop[:], 2.0, None, AOT.mult)
        rs8 = smp.tile([P, 1], dt.int8, tag="rs8")
        nc.vector.tensor_tensor(rs8[:], rsf[:], state[:], AOT.add)
        nc.sync.dma_start(d_rs[:], rs8[:])

    nc.compile()
    return nc


def _get_nc(n_tiles, f):
    key = (n_tiles, f)
    if key not in _cache:
        _cache[key] = _build(n_tiles, f)
    return _cache[key]


def run_sharded(a_u8, b_u8, n_cores=N_CORES, f=F, **spmd_kwargs):
    """Run the SPMD kernel over n_cores contiguous shards. Returns
    (out_u8_without_boundary_fixup, list[row_states int8[128]])."""
    from concourse import bass_utils

    n = a_u8.size
    words = n // 4
    wpc = words // n_cores
    n_tiles = wpc // (P * f)
    assert n_tiles * P * f == wpc, (n, n_cores, f)

    a32 = a_u8.view(np.uint32).reshape(n_cores, P, n_tiles * f)
    b32 = b_u8.view(np.uint32).reshape(n_cores, P, n_tiles * f)

    nc = _get_nc(n_tiles, f)
    in_maps = [{"a": np.ascontiguousarray(a32[c]),
                "b": np.ascontiguousarray(b32[c])}
               for c in range(n_cores)]
    res = bass_utils.run_bass_kernel_spmd(nc, in_maps,
                                          core_ids=list(range(n_cores)),
                                          **spmd_kwargs)
    outs = [r["o"] for r in res.results]
    rowstates = [r["rowst"].reshape(-1).astype(np.int8) for r in res.results]
    out = np.concatenate([o.reshape(-1).view(np.uint8) for o in outs])
    return out, rowstates, res


def _fixup_boundaries(out, a_u8, b_u8, rowstates, n_cores):
    """Resolve row-boundary carries on the host (decoupled lookback).

    Each row (core c, partition p) of ROW_BYTES bytes was computed with
    carry-in 0. Walk rows in stream order; when the true carry-in is 1,
    patch the row's all-propagate prefix (out = a|b) and bump the first
    non-propagate byte. Expected O(1) bytes of work per row.
    """
    carry = 0
    for c in range(n_cores):
        st = rowstates[c]
        for p in range(P):
            if carry:
                base = c * P * ROW_BYTES + p * ROW_BYTES
                i = base
                en = base + ROW_BYTES
                done = False
                while i < en and not done:
                    j = min(i + 4096, en)
                    aa = a_u8[i:j]
                    bb = b_u8[i:j]
                    raw = (_BREV[aa & bb].astype(np.int32)
                           + _BREV[bb].astype(np.int32))
                    prop = raw == 255
                    if prop.all():
                        out[i:j] = aa | bb
                        i = j
                        continue
                    k = int(np.argmin(prop))  # first non-propagate byte
                    out[i:i + k] = aa[:k] | bb[:k]
                    idx = i + k
                    new_s = (int(raw[k]) + 1) & 0xFF
                    out[idx] = ((int(_BREV[new_s]) ^ int(b_u8[idx]))
                                | int(a_u8[idx]))
                    done = True
            sv = int(st[p])
            carry = 1 if sv == 1 else (carry if sv == 2 else 0)
    return out


def kernel(a, b):
    assert a.dtype == np.uint8 and b.dtype == np.uint8 and a.size == N_BYTES
    out, rowstates, _ = run_sharded(a, b)
    out = _fixup_boundaries(out, a, b, rowstates, N_CORES)
    return out


# revision 18
# speedup vs baseline: 1.0342x; 1.0342x over previous
"""BSMatchStar Trainium2 kernel (v3, software-pipelined).

out = (a | (((a&b) +_brev b) ^ b))  -- bitstream MatchStar via a 2^29-bit
big-integer addition over per-byte bit-reversed operands.

Layout: per core, the 8 MiB byte-slice is uint32 words [128, 16384];
each partition owns a contiguous 16384-word chunk of the stream. Tiles
slice the free dim [128, F]. The limb-carry scan chains across tiles
via a [128,1] running state (seed 2.0 marks the all-propagate prefix),
so there is no cross-partition stitch on device; row-boundary carries
are patched on the host by walking the all-propagate prefix of each row
(expected ~1 byte per row).

Software pipeline: phase A(t) = load, a&b, brev x2, s = tr+br (gpsimd);
phase B(t) = flags, scan, carry apply (gpsimd), output brev, store.
Emitted as A(0) A(1) B(0) A(2) B(1) ... so the two mandatory gpsimd
adds always overlap DVE work from the neighboring phase.

Engines: DVE does all bitwise/shift work (gpsimd supports only int add
for uint32; CCE/ACT can't do bitwise). ACT computes the scan flags
p8 = relu(1-2*~s) and g01 = sign(msb-bit) off the DVE's back.
"""
import sys
sys.path.insert(0, "/opt/trn_rl_repo")

import numpy as np

N_BYTES = 67_108_864
N_CORES = 8
P = 128
F = 2048
WORDS_PER_CORE = N_BYTES // 4 // N_CORES   # 2,097,152
ROW_WORDS = WORDS_PER_CORE // P            # 16,384
T = ROW_WORDS // F                         # 8
ROW_BYTES = ROW_WORDS * 4                  # 65,536

_BREV = np.array([int(f"{i:08b}"[::-1], 2) for i in range(256)], dtype=np.uint8)

_cache = {}

ML = [0xF0F0F0F0, 0xCCCCCCCC, 0xAAAAAAAA]
MR = [0x0F0F0F0F, 0x33333333, 0x55555555]
SH = [4, 2, 1]


def _build(n_tiles, f):
    import concourse.bacc as bacc
    import concourse.tile as tile
    import concourse.mybir as mybir
    import contextlib

    AOT = mybir.AluOpType
    AFT = mybir.ActivationFunctionType
    dt = mybir.dt

    nc = bacc.Bacc("TRN2", target_bir_lowering=False, debug=False)

    nf = n_tiles * f
    d_a = nc.dram_tensor("a", [P, nf], dt.uint32, kind="ExternalInput")
    d_b = nc.dram_tensor("b", [P, nf], dt.uint32, kind="ExternalInput")
    d_o = nc.dram_tensor("o", [P, nf], dt.uint32, kind="ExternalOutput")
    d_rs = nc.dram_tensor("rowst", [P, 1], dt.int8, kind="ExternalOutput")

    with tile.TileContext(nc) as tc, contextlib.ExitStack() as ctx:
        pool = ctx.enter_context(tc.tile_pool(name="sb", bufs=1))
        iop = ctx.enter_context(tc.tile_pool(name="io", bufs=2))
        smp = ctx.enter_context(tc.tile_pool(name="sm", bufs=2))

        state0 = smp.tile([P, 1], dt.float32, tag="state")
        nc.vector.memset(state0[:], 0.0)
        allprop0 = smp.tile([P, 1], dt.float32, tag="allprop")
        nc.vector.memset(allprop0[:], 1.0)

        def stt(out, in0, imm, in1, op0, op1):
            nc.vector.add_instruction(
                mybir.InstTensorScalarPtr(
                    name=nc.get_next_instruction_name(),
                    is_scalar_tensor_tensor=True,
                    op0=op0, op1=op1,
                    ins=[nc.vector.lower_ap(in0),
                         mybir.ImmediateValue(dtype=dt.uint32, value=imm),
                         nc.vector.lower_ap(in1)],
                    outs=[nc.vector.lower_ap(out)],
                )
            )

        def brev(dst_tag, src, ts, dst_bufs=1, comb=None):
            x = src
            for li in range(3):
                A = pool.tile([P, f], dt.uint32, tag=f"{ts}A", name=f"{ts}A")
                B = pool.tile([P, f], dt.uint32, tag=f"{ts}B", name=f"{ts}B")
                nc.vector.tensor_scalar(A[:], x[:], SH[li], ML[li],
                                        AOT.logical_shift_left,
                                        AOT.bitwise_and)
                nc.vector.tensor_scalar(B[:], x[:], SH[li], MR[li],
                                        AOT.logical_shift_right,
                                        AOT.bitwise_and)
                y = pool.tile([P, f], dt.uint32,
                              tag=(dst_tag if li == 2 else f"{ts}Y"),
                              name=f"{ts}y{li}",
                              bufs=(dst_bufs if li == 2 else None))
                eng = comb[li] if comb else nc.vector
                op = AOT.add if eng is nc.gpsimd else AOT.bitwise_or
                eng.tensor_tensor(y[:], A[:], B[:], op)
                x = y
            return x

        ctxs = {}

        def phaseA(t):
            sl = slice(t * f, (t + 1) * f)
            a_t = iop.tile([P, f], dt.uint32, tag="a_t")
            b_t = iop.tile([P, f], dt.uint32, tag="b_t")
            nc.sync.dma_start(a_t[:], d_a[:, sl])
            nc.sync.dma_start(b_t[:], d_b[:, sl])

            t0 = pool.tile([P, f], dt.uint32, tag="vY", name="t0")
            nc.vector.tensor_tensor(t0[:], a_t[:], b_t[:], AOT.bitwise_and)

            tr = brev("tr", t0, "v", dst_bufs=2)
            br = brev("br", b_t, "u", dst_bufs=2)

            s = pool.tile([P, f], dt.uint32, tag="s", bufs=2)
            nc.gpsimd.tensor_tensor(s[:], tr[:], br[:], AOT.add)
            ctxs[t] = (a_t, tr, br, s)

        def phaseB(t, state, allprop):
            sl = slice(t * f, (t + 1) * f)
            a_t, tr, br, s = ctxs.pop(t)

            nots = pool.tile([P, f], dt.uint32, tag="nots")
            nc.vector.tensor_scalar(nots[:], s[:], 0xFFFFFFFF, None,
                                    AOT.bitwise_xor)
            # p8 = (nots == 0) via relu(1 - 2*nots) on the Scalar engine;
            # accum_out = row sum of p8 (for the all-propagate row flag)
            p8 = pool.tile([P, f], dt.int8, tag="p8")
            psum = smp.tile([P, 1], dt.float32, tag="psum")
            nc.scalar.activation(p8[:], nots[:], AFT.Relu,
                                 bias=1.0, scale=-2.0, accum_out=psum[:])
            # gmsb = msb & (tr | (br & ~s)), fused into two stt ops
            n1m = pool.tile([P, f], dt.uint32, tag="n1")
            stt(n1m[:], nots[:], 0x80000000, br[:],
                AOT.bitwise_and, AOT.bitwise_and)
            gmsb = pool.tile([P, f], dt.uint32, tag="g2")
            stt(gmsb[:], tr[:], 0x80000000, n1m[:],
                AOT.bitwise_and, AOT.bitwise_or)
            # g01 = sign(gmsb) in {0,1} on the Scalar engine
            g01 = pool.tile([P, f], dt.int8, tag="g01")
            nc.scalar.activation(g01[:], gmsb[:], AFT.Sign)

            # inclusive scan (uint32 out) into cols 1..f; col 0 = incoming
            # state, so lcb[:, 0:f] is the exclusive carry-in per limb,
            # directly usable as the +carry operand
            lcb = pool.tile([P, f + 1], dt.uint32, tag="lcb")
            nc.vector.tensor_copy(lcb[:, 0:1], state[:])
            nc.vector.tensor_tensor_scan(lcb[:, 1:f + 1], p8[:], g01[:],
                                         state[:], AOT.mult, AOT.max)
            nstate = smp.tile([P, 1], dt.float32, tag="state", name="nstate")
            nc.vector.tensor_copy(nstate[:], lcb[:, f:f + 1])

            # all-propagate row tracking: allprop *= (sum(p8) == f)
            rprop = smp.tile([P, 1], dt.float32, tag="rprop")
            nc.vector.tensor_scalar(rprop[:], psum[:], float(f), None,
                                    AOT.is_equal)
            nallprop = smp.tile([P, 1], dt.float32, tag="allprop",
                                name="nallprop")
            nc.vector.tensor_tensor(nallprop[:], allprop[:], rprop[:],
                                    AOT.mult)

            sp = pool.tile([P, f], dt.uint32, tag="nots", name="sp")
            nc.gpsimd.tensor_tensor(sp[:], s[:], lcb[:, 0:f], AOT.add)
            w = pool.tile([P, f], dt.uint32, tag="n1", name="w")
            nc.vector.tensor_tensor(w[:], sp[:], br[:], AOT.bitwise_xor)

            wb = brev("wY", w, "w")
            o_t = iop.tile([P, f], dt.uint32, tag="o_t", bufs=2)
            nc.vector.tensor_tensor(o_t[:], wb[:], a_t[:], AOT.bitwise_or)
            nc.sync.dma_start(d_o[:, sl], o_t[:])
            return nstate, nallprop

        state, allprop = state0, allprop0
        phaseA(0)
        for t in range(1, n_tiles):
            phaseA(t)
            state, allprop = phaseB(t - 1, state, allprop)
        state, allprop = phaseB(n_tiles - 1, state, allprop)

        # rowstate = carry-out {0,1} + 2*allprop (mutually exclusive)
        rsf = smp.tile([P, 1], dt.float32, tag="rsf")
        nc.vector.tensor_scalar(rsf[:], allprop[:], 2.0, None, AOT.mult)
        rs8 = smp.tile([P, 1], dt.int8, tag="rs8")
        nc.vector.tensor_tensor(rs8[:], rsf[:], state[:], AOT.add)
        nc.sync.dma_start(d_rs[:], rs8[:])

    nc.compile()
    return nc


def _get_nc(n_tiles, f):
    key = (n_tiles, f)
    if key not in _cache:
        _cache[key] = _build(n_tiles, f)
    return _cache[key]


def run_sharded(a_u8, b_u8, n_cores=N_CORES, f=F, **spmd_kwargs):
    """Run the SPMD kernel over n_cores contiguous shards. Returns
    (out_u8_without_boundary_fixup, list[row_states int8[128]])."""
    from concourse import bass_utils

    n = a_u8.size
    words = n // 4
    wpc = words // n_cores
    n_tiles = wpc // (P * f)
    assert n_tiles * P * f == wpc, (n, n_cores, f)

    a32 = a_u8.view(np.uint32).reshape(n_cores, P, n_tiles * f)
    b32 = b_u8.view(np.uint32).reshape(n_cores, P, n_tiles * f)

    nc = _get_nc(n_tiles, f)
    in_maps = [{"a": np.ascontiguousarray(a32[c]),
                "b": np.ascontiguousarray(b32[c])}
               for c in range(n_cores)]
    res = bass_utils.run_bass_kernel_spmd(nc, in_maps,
                                          core_ids=list(range(n_cores)),
                                          **spmd_kwargs)
    outs = [r["o"] for r in res.results]
    rowstates = [r["rowst"].reshape(-1).astype(np.int8) for r in res.results]
    out = np.concatenate([o.reshape(-1).view(np.uint8) for o in outs])
    return out, rowstates, res


def _fixup_boundaries(out, a_u8, b_u8, rowstates, n_cores):
    """Resolve row-boundary carries on the host (decoupled lookback).

    Each row (core c, partition p) of ROW_BYTES bytes was computed with
    carry-in 0. Walk rows in stream order; when the true carry-in is 1,
    patch the row's all-propagate prefix (out = a|b) and bump the first
    non-propagate byte. Expected O(1) bytes of work per row.
    """
    carry = 0
    for c in range(n_cores):
        st = rowstates[c]
        for p in range(P):
            if carry:
                base = c * P * ROW_BYTES + p * ROW_BYTES
                i = base
                en = base + ROW_BYTES
                done = False
                while i < en and not done:
                    j = min(i + 4096, en)
                    aa = a_u8[i:j]
                    bb = b_u8[i:j]
                    raw = (_BREV[aa & bb].astype(np.int32)
                           + _BREV[bb].astype(np.int32))
                    prop = raw == 255
                    if prop.all():
                        out[i:j] = aa | bb
                        i = j
                        continue
                    k = int(np.argmin(prop))  # first non-propagate byte
                    out[i:i + k] = aa[:k] | bb[:k]
                    idx = i + k
                    new_s = (int(raw[k]) + 1) & 0xFF
                    out[idx] = ((int(_BREV[new_s]) ^ int(b_u8[idx]))
                                | int(a_u8[idx]))
                    done = True
            sv = int(st[p])
            carry = 1 if sv == 1 else (carry if sv == 2 else 0)
    return out


def kernel(a, b):
    assert a.dtype == np.uint8 and b.dtype == np.uint8 and a.size == N_BYTES
    out, rowstates, _ = run_sharded(a, b)
    out = _fixup_boundaries(out, a, b, rowstates, N_CORES)
    return out


# revision 24
# speedup vs baseline: 1.1180x; 1.0810x over previous
"""BSMatchStar Trainium2 kernel (v3, software-pipelined).

out = (a | (((a&b) +_brev b) ^ b))  -- bitstream MatchStar via a 2^29-bit
big-integer addition over per-byte bit-reversed operands.

Layout: per core, the 8 MiB byte-slice is uint32 words [128, 16384];
each partition owns a contiguous 16384-word chunk of the stream. Tiles
slice the free dim [128, F]. The limb-carry scan chains across tiles
via a [128,1] running state (seed 2.0 marks the all-propagate prefix),
so there is no cross-partition stitch on device; row-boundary carries
are patched on the host by walking the all-propagate prefix of each row
(expected ~1 byte per row).

Software pipeline: phase A(t) = load, a&b, brev x2, s = tr+br (gpsimd);
phase B(t) = flags, scan, carry apply (gpsimd), output brev, store.
Emitted as A(0) A(1) B(0) A(2) B(1) ... so the two mandatory gpsimd
adds always overlap DVE work from the neighboring phase.

Engines: DVE does all bitwise/shift work (gpsimd supports only int add
for uint32; CCE/ACT can't do bitwise). ACT computes the scan flags
p8 = relu(1-2*~s) and g01 = sign(msb-bit) off the DVE's back.
"""
import sys
sys.path.insert(0, "/opt/trn_rl_repo")

import numpy as np

N_BYTES = 67_108_864
N_CORES = 8
P = 128
F = 2048
WORDS_PER_CORE = N_BYTES // 4 // N_CORES   # 2,097,152
ROW_WORDS = WORDS_PER_CORE // P            # 16,384
T = ROW_WORDS // F                         # 8
ROW_BYTES = ROW_WORDS * 4                  # 65,536

_BREV = np.array([int(f"{i:08b}"[::-1], 2) for i in range(256)], dtype=np.uint8)

_cache = {}

ML = [0xF0F0F0F0, 0xCCCCCCCC, 0xAAAAAAAA]
MR = [0x0F0F0F0F, 0x33333333, 0x55555555]
SH = [4, 2, 1]


def _build(n_tiles, f):
    import concourse.bacc as bacc
    import concourse.tile as tile
    import concourse.mybir as mybir
    import contextlib

    AOT = mybir.AluOpType
    AFT = mybir.ActivationFunctionType
    dt = mybir.dt

    nc = bacc.Bacc("TRN2", target_bir_lowering=False, debug=False)

    nf = n_tiles * f
    d_a = nc.dram_tensor("a", [P, nf], dt.uint32, kind="ExternalInput")
    d_b = nc.dram_tensor("b", [P, nf], dt.uint32, kind="ExternalInput")
    d_o = nc.dram_tensor("o", [P, nf], dt.uint32, kind="ExternalOutput")
    d_rs = nc.dram_tensor("rowst", [P, 1], dt.int8, kind="ExternalOutput")
    d_rp = nc.dram_tensor("rowps", [P, 1], dt.float32, kind="ExternalOutput")

    with tile.TileContext(nc) as tc, contextlib.ExitStack() as ctx:
        pool = ctx.enter_context(tc.tile_pool(name="sb", bufs=1))
        iop = ctx.enter_context(tc.tile_pool(name="io", bufs=2))
        smp = ctx.enter_context(tc.tile_pool(name="sm", bufs=2))

        state0 = smp.tile([P, 1], dt.float32, tag="state")
        nc.vector.memset(state0[:], 0.0)
        rpsum0 = smp.tile([P, 1], dt.float32, tag="rpsum")
        nc.vector.memset(rpsum0[:], 0.0)

        def stt(out, in0, imm, in1, op0, op1):
            nc.vector.add_instruction(
                mybir.InstTensorScalarPtr(
                    name=nc.get_next_instruction_name(),
                    is_scalar_tensor_tensor=True,
                    op0=op0, op1=op1,
                    ins=[nc.vector.lower_ap(in0),
                         mybir.ImmediateValue(dtype=dt.uint32, value=imm),
                         nc.vector.lower_ap(in1)],
                    outs=[nc.vector.lower_ap(out)],
                )
            )

        def brev(dst_tag, src, ts, dst_bufs=1, comb=None):
            x = src
            for li in range(3):
                A = pool.tile([P, f], dt.uint32, tag=f"{ts}A", name=f"{ts}A")
                B = pool.tile([P, f], dt.uint32, tag=f"{ts}B", name=f"{ts}B")
                nc.vector.tensor_scalar(A[:], x[:], SH[li], ML[li],
                                        AOT.logical_shift_left,
                                        AOT.bitwise_and)
                nc.vector.tensor_scalar(B[:], x[:], SH[li], MR[li],
                                        AOT.logical_shift_right,
                                        AOT.bitwise_and)
                y = pool.tile([P, f], dt.uint32,
                              tag=(dst_tag if li == 2 else f"{ts}Y"),
                              name=f"{ts}y{li}",
                              bufs=(dst_bufs if li == 2 else None))
                eng = comb[li] if comb else nc.vector
                op = AOT.add if eng is nc.gpsimd else AOT.bitwise_or
                eng.tensor_tensor(y[:], A[:], B[:], op)
                x = y
            return x

        ctxs = {}

        def phaseA(t):
            sl = slice(t * f, (t + 1) * f)
            a_t = iop.tile([P, f], dt.uint32, tag="a_t")
            b_t = iop.tile([P, f], dt.uint32, tag="b_t")
            nc.sync.dma_start(a_t[:], d_a[:, sl])
            nc.sync.dma_start(b_t[:], d_b[:, sl])

            t0 = pool.tile([P, f], dt.uint32, tag="vY", name="t0")
            nc.vector.tensor_tensor(t0[:], a_t[:], b_t[:], AOT.bitwise_and)

            tr = brev("tr", t0, "v", dst_bufs=2)
            br = brev("br", b_t, "u", dst_bufs=2)

            s = pool.tile([P, f], dt.uint32, tag="s", bufs=2)
            nc.gpsimd.tensor_tensor(s[:], tr[:], br[:], AOT.add)
            ctxs[t] = (a_t, tr, br, s)

        def phaseB(t, state, rpsum):
            sl = slice(t * f, (t + 1) * f)
            a_t, tr, br, s = ctxs.pop(t)

            nots = pool.tile([P, f], dt.uint32, tag="nots")
            nc.vector.tensor_scalar(nots[:], s[:], 0xFFFFFFFF, None,
                                    AOT.bitwise_xor)
            # p8 = (nots == 0) via relu(1 - 2*nots) on the Scalar engine.
            # Propagate limbs need a==0 && b==0xFFFFFFFF (P ~= 2^-64), so we
            # only use the row SUM (accum_out): rows with any propagate limb
            # are recomputed exactly on the host; no on-device scan needed.
            p8 = pool.tile([P, f], dt.int8, tag="p8")
            psum = smp.tile([P, 1], dt.float32, tag="psum")
            nc.scalar.activation(p8[:], nots[:], AFT.Relu,
                                 bias=1.0, scale=-2.0, accum_out=psum[:])
            # gmsb = msb & (tr | (br & ~s)), fused into two stt ops
            n1m = pool.tile([P, f], dt.uint32, tag="n1")
            stt(n1m[:], nots[:], 0x80000000, br[:],
                AOT.bitwise_and, AOT.bitwise_and)
            gmsb = pool.tile([P, f], dt.uint32, tag="g2")
            stt(gmsb[:], tr[:], 0x80000000, n1m[:],
                AOT.bitwise_and, AOT.bitwise_or)
            # With no propagate limbs, carry-in[j] = g01[j-1]: write
            # g01 = sign(gmsb) (Scalar engine, uint32 out) directly into the
            # shifted carry buffer; col 0 = incoming cross-tile state.
            gbuf = pool.tile([P, f + 1], dt.uint32, tag="lcb")
            nc.vector.tensor_copy(gbuf[:, 0:1], state[:])
            nc.scalar.activation(gbuf[:, 1:f + 1], gmsb[:], AFT.Sign)
            nstate = smp.tile([P, 1], dt.float32, tag="state", name="nstate")
            nc.vector.tensor_copy(nstate[:], gbuf[:, f:f + 1])
            nrp = smp.tile([P, 1], dt.float32, tag="rpsum", name="nrp")
            nc.vector.tensor_tensor(nrp[:], rpsum[:], psum[:], AOT.add)

            sp = pool.tile([P, f], dt.uint32, tag="nots", name="sp")
            nc.gpsimd.tensor_tensor(sp[:], s[:], gbuf[:, 0:f], AOT.add)
            w = pool.tile([P, f], dt.uint32, tag="n1", name="w")
            nc.vector.tensor_tensor(w[:], sp[:], br[:], AOT.bitwise_xor)

            wb = brev("wY", w, "w")
            o_t = iop.tile([P, f], dt.uint32, tag="o_t", bufs=2)
            nc.vector.tensor_tensor(o_t[:], wb[:], a_t[:], AOT.bitwise_or)
            nc.sync.dma_start(d_o[:, sl], o_t[:])
            return nstate, nrp

        state, rpsum = state0, rpsum0
        phaseA(0)
        for t in range(1, n_tiles):
            phaseA(t)
            state, rpsum = phaseB(t - 1, state, rpsum)
        state, rpsum = phaseB(n_tiles - 1, state, rpsum)

        rs8 = smp.tile([P, 1], dt.int8, tag="rs8")
        nc.vector.tensor_copy(rs8[:], state[:])
        nc.sync.dma_start(d_rs[:], rs8[:])
        nc.sync.dma_start(d_rp[:], rpsum[:])

    nc.compile()
    return nc


def _get_nc(n_tiles, f):
    key = (n_tiles, f)
    if key not in _cache:
        _cache[key] = _build(n_tiles, f)
    return _cache[key]


def run_sharded(a_u8, b_u8, n_cores=N_CORES, f=F, **spmd_kwargs):
    """Run the SPMD kernel over n_cores contiguous shards. Returns
    (out_u8_without_boundary_fixup, list[row_states int8[128]])."""
    from concourse import bass_utils

    n = a_u8.size
    words = n // 4
    wpc = words // n_cores
    n_tiles = wpc // (P * f)
    assert n_tiles * P * f == wpc, (n, n_cores, f)

    a32 = a_u8.view(np.uint32).reshape(n_cores, P, n_tiles * f)
    b32 = b_u8.view(np.uint32).reshape(n_cores, P, n_tiles * f)

    nc = _get_nc(n_tiles, f)
    in_maps = [{"a": np.ascontiguousarray(a32[c]),
                "b": np.ascontiguousarray(b32[c])}
               for c in range(n_cores)]
    res = bass_utils.run_bass_kernel_spmd(nc, in_maps,
                                          core_ids=list(range(n_cores)),
                                          **spmd_kwargs)
    outs = [r["o"] for r in res.results]
    rowinfo = [(r["rowst"].reshape(-1).astype(np.int8),
                r["rowps"].reshape(-1).astype(np.float32))
               for r in res.results]
    out = np.concatenate([o.reshape(-1).view(np.uint8) for o in outs])
    return out, rowinfo, res


def _fixup_boundaries(out, a_u8, b_u8, rowinfo, n_cores):
    """Resolve row-boundary carries on the host (decoupled lookback).

    Each row (core c, partition p) was computed with carry-in 0 AND with
    the assumption that no limb propagates (carry-in[j] = g01[j-1]).
    Rows with any propagate limb (rowps != 0; probability ~2^-64 per
    limb) are recomputed exactly here. For the rest, when the true
    row carry-in is 1, patch the all-propagate byte prefix (out = a|b)
    and bump the first non-propagate byte. Expected O(1) work per row.
    """
    carry = 0
    for c in range(n_cores):
        rs, rp = rowinfo[c]
        for p in range(P):
            base = c * P * ROW_BYTES + p * ROW_BYTES
            if rp[p] != 0:
                # exact byte-level recompute of this row with carry-in
                aa = a_u8[base:base + ROW_BYTES]
                bb = b_u8[base:base + ROW_BYTES]
                raw = (_BREV[aa & bb].astype(np.int32)
                       + _BREV[bb].astype(np.int32))
                cm = (raw & 255).astype(np.uint8)
                gen = (raw >> 8).astype(np.uint8)
                prop = cm == 255
                idx = np.arange(ROW_BYTES, dtype=np.int64)
                lastnp = np.maximum.accumulate(np.where(~prop, idx, -1))
                le = np.empty_like(lastnp)
                le[0] = -1
                le[1:] = lastnp[:-1]
                ci = np.where(le >= 0, gen[np.clip(le, 0, None)],
                              carry).astype(np.uint8)
                out[base:base + ROW_BYTES] = (
                    (_BREV[(cm + ci).astype(np.uint8)] ^ bb) | aa)
                ln = int(lastnp[-1])
                carry = int(gen[ln]) if ln >= 0 else carry
            else:
                if carry:
                    i = base
                    en = base + ROW_BYTES
                    done = False
                    while i < en and not done:
                        j = min(i + 4096, en)
                        aa = a_u8[i:j]
                        bb = b_u8[i:j]
                        raw = (_BREV[aa & bb].astype(np.int32)
                               + _BREV[bb].astype(np.int32))
                        prop = raw == 255
                        if prop.all():
                            out[i:j] = aa | bb
                            i = j
                            continue
                        k = int(np.argmin(prop))  # first non-propagate byte
                        out[i:i + k] = aa[:k] | bb[:k]
                        idx = i + k
                        new_s = (int(raw[k]) + 1) & 0xFF
                        out[idx] = ((int(_BREV[new_s]) ^ int(b_u8[idx]))
                                    | int(a_u8[idx]))
                        done = True
                carry = int(rs[p])
    return out


def kernel(a, b):
    assert a.dtype == np.uint8 and b.dtype == np.uint8 and a.size == N_BYTES
    out, rowstates, _ = run_sharded(a, b)
    out = _fixup_boundaries(out, a, b, rowstates, N_CORES)
    return out
